# revision 11
# baseline (speedup 1.0000x reference)
"""AdaAug augmentation pipeline on 8 TRN2 NeuronCores (pure data parallel).

Pipeline per sample: color transform (3x3 + bias) -> 43-tap separable wavelet
filter with reflect padding -> additive RGB noise -> cutout mask.

Strategy:
  - Host derives per-sample small parameters exactly as the reference does:
    color matrix M3/b, filter taps hz', cutout mask vectors, sigma.
  - The separable conv with reflect padding is expressed as two chained PE
    matmuls per channel using a per-sample 256x256 reflect-Toeplitz matrix W:
        out1 = img.T @ W   ([w, h'] layout, contracts h)
        out2 = out1.T @ W  ([h', w'] layout, contracts w)
    No transposes needed: using the data as lhsT flips the layout each stage.
  - The 3x3 channel mix is folded into stage 2 by scaling W with M_cc' and
    accumulating over c' in PSUM (WhS built on the otherwise idle GPSIMD).
    The color bias reduces to a per-channel constant b_c * (sum k)^2, applied
    in the final fused DVE op. Noise is added via sigma*I identity matmuls.
  - Final op per channel: out = (psum + bias_c) * cutout_mask, one fused
    scalar_tensor_tensor DVE op, PSUM -> SBUF, then DMA out.
  - All bulk tensors are packed partition-major on the host so every DMA is
    128 contiguous per-partition chunks.
"""

import os
import sys

import numpy as np

if "/opt/trn_rl_repo" not in sys.path:
    sys.path.insert(0, "/opt/trn_rl_repo")

import ml_dtypes

N, C, H, W = 64, 3, 256, 256
NCORES = 8
NLOC = N // NCORES
TAP, PAD = 43, 21
PI = float(np.pi)
BRIGHT_STD, CONTRAST_STD, HUE_MAX, SAT_STD = 0.2, 0.5, 1.0, 1.0
IMGFILTER_STD, NOISE_STD, CUTOUT_SIZE = 1.0, 0.1, 0.5
P_GATE = 1.0
SM = 24  # per-sample slot count in the smalls tensor

BF16 = ml_dtypes.bfloat16


# --------------------------------------------------------------------------
# Host-side per-sample parameter derivation (mirrors the reference math)
# --------------------------------------------------------------------------

def color_matrices(gates, gauss, unif):
    """Returns M3 [n,3,3] and bvec [n,3] (float64)."""
    g = gates.astype(np.float64)
    ga = gauss.astype(np.float64)
    u = unif.astype(np.float64)
    n = g.shape[0]
    I4 = np.eye(4)
    inv_sqrt3 = 1.0 / np.sqrt(3.0)
    v3 = np.full(3, inv_sqrt3)
    v4 = np.array([inv_sqrt3, inv_sqrt3, inv_sqrt3, 0.0])
    vv = np.outer(v4, v4)

    b = np.where(g[:, 0] < P_GATE, ga[:, 0] * BRIGHT_STD, 0.0)
    T = np.broadcast_to(I4, (n, 4, 4)).copy()
    T[:, :3, 3] = b[:, None]

    c = np.where(g[:, 1] < P_GATE, 2.0 ** (ga[:, 1] * CONTRAST_STD), 1.0)
    S = I4[None] * np.stack([c, c, c, np.ones_like(c)], axis=1)[:, :, None]
    Cm = S @ T

    i_lf = np.floor(u[:, 0] * 2.0)
    i_lf = np.where(g[:, 2] < P_GATE, i_lf, 0.0)
    Cm = (I4[None] - 2.0 * vv[None] * i_lf[:, None, None]) @ Cm

    theta = (u[:, 1] * 2.0 - 1.0) * PI * HUE_MAX
    theta = np.where(g[:, 3] < P_GATE, theta, 0.0)
    I3 = np.eye(3)
    K = np.array([[0.0, -inv_sqrt3, inv_sqrt3],
                  [inv_sqrt3, 0.0, -inv_sqrt3],
                  [-inv_sqrt3, inv_sqrt3, 0.0]])
    co, si = np.cos(theta), np.sin(theta)
    R3 = ((1.0 - co)[:, None, None] * np.outer(v3, v3)[None]
          + co[:, None, None] * I3[None] + si[:, None, None] * K[None])
    R4 = np.broadcast_to(I4, (n, 4, 4)).copy()
    R4[:, :3, :3] = R3
    Cm = R4 @ Cm

    s = np.where(g[:, 4] < P_GATE, 2.0 ** (ga[:, 2] * SAT_STD), 1.0)
    Cm = (vv[None] + (I4 - vv)[None] * s[:, None, None]) @ Cm
    return Cm[:, :3, :3], Cm[:, :3, 3]


def band_taps(gates, gauss, hz_fbank):
    """Returns hz' [n, 43] (float64)."""
    g = gates.astype(np.float64)
    ga = gauss.astype(np.float64)
    fb = hz_fbank.astype(np.float64)
    n = g.shape[0]
    num_bands = fb.shape[0]
    ep = np.array([10.0, 1.0, 1.0, 1.0]) / 13.0
    gg = np.ones((n, num_bands))
    for i in range(num_bands):
        t_i = 2.0 ** (ga[:, 3 + i] * IMGFILTER_STD)
        t_i = np.where(g[:, 5 + i] < P_GATE, t_i, 1.0)
        t = np.ones((n, num_bands))
        t[:, i] = t_i
        t = t / np.sqrt(np.sum(ep * t * t, axis=-1, keepdims=True))
        gg = gg * t
    return gg @ fb


def toeplitz_reflect(k):
    """W [256,256] such that (reflect-pad-21 conv k) == W.T @ x.  k: [43]."""
    Wm = np.zeros((H, H))
    j = np.arange(H)
    for t in range(TAP):
        m = j + t - PAD
        m = np.abs(m)
        m = np.where(m > H - 1, 2 * (H - 1) - m, m)
        np.add.at(Wm, (m, j), k[t])
    return Wm


def mask_vectors(gates, unif):
    """Exact f32 cutout indicator vectors mx, my: [n, 256] each (1.0 outside)."""
    g32 = gates.astype(np.float32)
    u32 = unif.astype(np.float32)
    size = np.where(g32[:, 10] < np.float32(P_GATE),
                    np.float32(CUTOUT_SIZE), np.float32(0.0)).astype(np.float32)
    half = (size * np.float32(0.5)).astype(np.float32)
    coord = ((np.arange(W, dtype=np.float32) + np.float32(0.5))
             / np.float32(W)).astype(np.float32)
    cx, cy = u32[:, 2], u32[:, 3]
    mx = (np.abs(coord[None, :] - cx[:, None]) >= half[:, None]).astype(np.float32)
    my = (np.abs(coord[None, :] - cy[:, None]) >= half[:, None]).astype(np.float32)
    return mx, my


def derive_params(gates, gauss, unif, hz_fbank):
    """All per-sample derived parameters for the device kernel."""
    n = gates.shape[0]
    M3, bvec = color_matrices(gates, gauss, unif)
    hz = band_taps(gates, gauss, hz_fbank)
    sk = hz.sum(axis=1)
    bias_c = bvec * (sk ** 2)[:, None]  # [n, 3]
    g32 = gates.astype(np.float32)
    sigma = np.where(g32[:, 9] < np.float32(P_GATE),
                     np.abs(gauss[:, 7].astype(np.float32)) * np.float32(NOISE_STD),
                     np.float32(0.0))
    mx, my = mask_vectors(gates, unif)
    Wmats = np.stack([toeplitz_reflect(hz[s]) for s in range(n)])
    return dict(M3=M3, bias_c=bias_c, sigma=sigma, mx=mx, my=my, Wmats=Wmats)


def pack_smalls(M3, bias_c, sigma, my):
    """[128, n*SM] f32, partition-replicated scalars + partition-indexed my."""
    n = M3.shape[0]
    sm = np.zeros((128, n * SM), dtype=np.float32)
    for s in range(n):
        base = s * SM
        sm[:, base:base + 9] = M3[s].reshape(-1).astype(np.float32)[None, :]
        sm[:, base + 9] = np.float32(sigma[s])
        sm[:, base + 10:base + 13] = bias_c[s].astype(np.float32)[None, :]
        for t in range(2):
            sm[:, base + 13 + t] = my[s][t * 128:(t + 1) * 128]
    return sm


def pack_images(x):
    """[n, 3, 256, 256] -> partition-major [n, 128, 1536]:
    buf[s, p, c*512 + t*256 + w] = x[s, c, t*128 + p, w]."""
    n = x.shape[0]
    return np.ascontiguousarray(
        x.reshape(n, C, 2, 128, W).transpose(0, 3, 1, 2, 4).reshape(n, 128, C * 2 * W)
    )


def unpack_images(buf):
    """Inverse of pack_images (for the f32 output)."""
    n = buf.shape[0]
    return np.ascontiguousarray(
        buf.reshape(n, 128, C, 2, W).transpose(0, 2, 3, 1, 4).reshape(n, C, H, W)
    )


def pack_wmats(Wm):
    """[n, 256, 256] -> [n, 128, 512]: buf[s, p, t*256 + j] = W[s, t*128+p, j]."""
    n = Wm.shape[0]
    return np.ascontiguousarray(
        Wm.reshape(n, 2, 128, H).transpose(0, 2, 1, 3).reshape(n, 128, 2 * H)
    )


# --------------------------------------------------------------------------
# Pure-numpy emulation of the device pipeline (for host-side validation)
# --------------------------------------------------------------------------

def emulate(images, noise_img, params, f32_sim=True):
    """Emulates the device computation (in f64 or with bf16 rounding)."""
    M3, bias_c, sigma = params["M3"], params["bias_c"], params["sigma"]
    mx, my, Wmats = params["mx"], params["my"], params["Wmats"]
    n = images.shape[0]
    out = np.zeros((n, C, H, W), dtype=np.float64)
    for s in range(n):
        Wm = Wmats[s]
        img = images[s].astype(np.float64)
        st1 = np.tensordot(img, Wm, axes=([1], [0]))  # [c, w, h']
        conv = np.tensordot(st1, Wm, axes=([1], [0]))  # [c, h', w']
        mixed = np.tensordot(M3[s], conv, axes=([1], [0]))
        mixed += bias_c[s][:, None, None]
        mixed += sigma[s] * noise_img[s].astype(np.float64)
        mask = np.maximum(my[s][:, None], mx[s][None, :])
        out[s] = mixed * mask[None]
    return out


# --------------------------------------------------------------------------
# Bass kernel builder
# --------------------------------------------------------------------------

def _legalize_waits(nc, max_keep=1):
    """Split multi-semaphore waits into standalone EventSemaphore instructions.

    The deployed walrus accepts at most one sync-wait command per engine
    instruction; Tile emits several. Hoisting extras onto preceding
    EventSemaphore instructions on the same engine queue is semantically
    identical (engines execute their stream in order)."""
    from concourse import mybir
    n_split = 0
    for f in nc.m.functions:
        for blk in f.blocks:
            out = []
            changed = False
            for inst in blk.instructions:
                si = inst.sync_info
                w = list(si.on_wait) if si is not None else []
                if len(w) > max_keep:
                    for extra in w[:-max_keep]:
                        ev = mybir.InstEventSemaphore(
                            name=f"evw_{n_split}", ins=[], outs=[])
                        ev.engine = inst.engine
                        ev.sync_info = mybir.SyncInfo(
                            on_wait=[extra], on_update=[])
                        out.append(ev)
                        n_split += 1
                    inst.sync_info = mybir.SyncInfo(
                        on_wait=w[-max_keep:], on_update=list(si.on_update))
                    changed = True
                out.append(inst)
            if changed:
                blk.instructions = out
    return nc


def _ap_key(arg, extras=()):
    """Identity key for a lowered matmul weights AP."""
    try:
        return (arg.memref, arg.offset, str(arg.ap), str(arg.dtype)) + tuple(
            str(e) for e in extras)
    except AttributeError:
        return None


def _dedupe_ldweights(nc):
    """Drop InstLdweights whose weights AP is identical to the previous weight
    load in the final PE stream (only matmuls/waits in between). The PE array
    already holds those weights; sem waits/updates are preserved on a
    standalone EventSemaphore."""
    from concourse import mybir
    n_removed = 0
    for f in nc.m.functions:
        for blk in f.blocks:
            out = []
            changed = False
            last_key = None
            for inst in blk.instructions:
                if inst.engine == mybir.EngineType.PE:
                    if isinstance(inst, mybir.InstLdweights):
                        key = _ap_key(
                            inst.ins[0],
                            extras=(inst.perf_mode, inst.is_transpose,
                                    inst.tile_position),
                        )
                        if key is not None and key == last_key:
                            si = inst.sync_info
                            if si is not None and (list(si.on_wait)
                                                   or list(si.on_update)):
                                ev = mybir.InstEventSemaphore(
                                    name=f"ldw_ev_{n_removed}", ins=[], outs=[])
                                ev.engine = inst.engine
                                ev.sync_info = si
                                out.append(ev)
                            n_removed += 1
                            changed = True
                            continue
                        last_key = key
                    elif isinstance(inst, mybir.InstMatmult):
                        if inst.ldweights:
                            last_key = None  # self-loading matmul clobbers
                    elif isinstance(inst, mybir.InstEventSemaphore):
                        pass  # does not touch the PE array
                    else:
                        last_key = None  # unknown PE inst: invalidate
                out.append(inst)
            if changed:
                blk.instructions = out
    return n_removed


def build_bass(legalize=True, dedupe_ldw=True):
    import concourse.bass as bass
    import concourse.tile as tile
    from concourse import mybir

    f32 = mybir.dt.float32
    bf16 = mybir.dt.bfloat16
    Alu = mybir.AluOpType
    Act = mybir.ActivationFunctionType

    nc = bass.Bass()
    d_img = nc.declare_dram_parameter("imgs", [NLOC, 128, C * 2 * W], bf16,
                                      isOutput=False)
    d_noi = nc.declare_dram_parameter("noise", [NLOC, 128, C * 2 * W], bf16,
                                      isOutput=False)
    d_w = nc.declare_dram_parameter("wmat", [NLOC, 128, 2 * H], bf16,
                                    isOutput=False)
    d_mask = nc.declare_dram_parameter("maskf", [NLOC, 128, 2 * W], bf16,
                                       isOutput=False)
    d_sm = nc.declare_dram_parameter("smalls", [128, NLOC * SM], f32,
                                     isOutput=False)
    d_out = nc.declare_dram_parameter("out", [NLOC, 128, C * 2 * W], bf16,
                                      isOutput=True)

    with tile.TileContext(nc) as tc:
        with (
            tc.tile_pool(name="singles", bufs=1) as singles,
            tc.tile_pool(name="io", bufs=4) as io,
            tc.tile_pool(name="work", bufs=3) as work,
            tc.tile_pool(name="ps1", bufs=4, space="PSUM") as ps1p,
            tc.tile_pool(name="ps2", bufs=4, space="PSUM") as ps2p,
        ):
            scal = singles.tile([128, NLOC * SM], f32)
            nc.sync.dma_start(out=scal, in_=d_sm[:])

            def sc(s, k):
                return scal[:, s * SM + k: s * SM + k + 1]

            for s in range(NLOC):
                # ---- loads (all contiguous per partition) ----
                img_sb = io.tile([128, C * 2 * W], bf16, tag="img")
                nc.sync.dma_start(out=img_sb, in_=d_img[s])
                noi_sb = io.tile([128, C * 2 * W], bf16, tag="noi")
                nc.sync.dma_start(out=noi_sb, in_=d_noi[s])
                w_sb = io.tile([128, 2 * H], bf16, tag="wm")
                nc.sync.dma_start(out=w_sb, in_=d_w[s])
                maskS = io.tile([128, 2 * W], bf16, tag="mask")
                nc.sync.dma_start(out=maskS, in_=d_mask[s])

                # ---- stage 1: vertical conv, per input channel ----
                # psum1_cp[p, m*256 + h'] = sum_h img[cp, h, w=m*128+p] * W[h, h']
                ps1_t = [
                    ps1p.tile([128, 2 * H], mybir.dt.float32, tag="ps1",
                              name=f"ps1_{cp}")
                    for cp in range(C)
                ]
                for cp in range(C):
                    pt = ps1_t[cp]
                    for m in range(2):
                        for k in range(2):
                            nc.tensor.matmul(
                                pt[:, m * 256:(m + 1) * 256],
                                img_sb[:, cp * 512 + k * 256 + m * 128:
                                       cp * 512 + k * 256 + m * 128 + 128],
                                w_sb[:, k * 256:(k + 1) * 256],
                                start=(k == 0),
                                stop=(k == 1),
                            )
                out1 = work.tile([128, C * 2 * H], bf16, tag="out1")
                for cp in range(C):
                    nc.scalar.copy(out1[:, cp * 512:(cp + 1) * 512], ps1_t[cp])

                # ---- stage 2: horizontal conv + channel mix + noise ----
                ps2_t = [
                    ps2p.tile([128, 2 * W], mybir.dt.float32, tag="ps2",
                              name=f"ps2_{cp}")
                    for cp in range(C)
                ]
                for cp in range(C):
                    for m in range(2):
                        for k in range(2):
                            nc.tensor.matmul(
                                ps2_t[cp][:, m * 256:(m + 1) * 256],
                                out1[:, cp * 512 + k * 256 + m * 128:
                                     cp * 512 + k * 256 + m * 128 + 128],
                                w_sb[:, k * 256:(k + 1) * 256],
                                start=(k == 0),
                                stop=(k == 1),
                            )
                conv_sb = work.tile([128, C * 2 * W], bf16, tag="conv")
                for cp in range(C):
                    nc.scalar.copy(conv_sb[:, cp * 512:(cp + 1) * 512],
                                   ps2_t[cp][:])

                # ---- mix + noise + bias + mask on DVE (bf16 chain) ----
                outS = work.tile([128, C * 2 * W], bf16, tag="outS")
                for c in range(C):
                    t = work.tile([128, 2 * W], bf16, tag="mixt",
                                  name=f"mixt_{c}")
                    nc.vector.tensor_scalar_mul(
                        t[:], conv_sb[:, 0:512], sc(s, c * 3 + 0))
                    for cp in (1, 2):
                        nc.vector.scalar_tensor_tensor(
                            out=t[:],
                            in0=conv_sb[:, cp * 512:(cp + 1) * 512],
                            scalar=sc(s, c * 3 + cp),
                            in1=t[:],
                            op0=Alu.mult,
                            op1=Alu.add,
                        )
                    nc.vector.scalar_tensor_tensor(
                        out=t[:],
                        in0=noi_sb[:, c * 512:(c + 1) * 512],
                        scalar=sc(s, 9),
                        in1=t[:],
                        op0=Alu.mult,
                        op1=Alu.add,
                    )
                    nc.vector.scalar_tensor_tensor(
                        out=outS[:, c * 512:(c + 1) * 512],
                        in0=t[:],
                        scalar=sc(s, 10 + c),
                        in1=maskS[:],
                        op0=Alu.add,
                        op1=Alu.mult,
                    )
                nc.sync.dma_start(out=d_out[s], in_=outS)
    if dedupe_ldw:
        n = _dedupe_ldweights(nc)
        if os.environ.get("ADAAUG_DEBUG"):
            print(f"deduped {n} LDWEIGHTS")
    return _legalize_waits(nc) if legalize else nc


# --------------------------------------------------------------------------
# Entry point
# --------------------------------------------------------------------------

def _prep_in_maps(images, gates, gauss, unif, noise_img, hz_fbank):
    prm = derive_params(gates, gauss, unif, hz_fbank)
    imgs_bf = pack_images(images).astype(BF16)
    noise_bf = pack_images(noise_img).astype(BF16)
    w_bf = pack_wmats(prm["Wmats"].astype(np.float32)).astype(BF16)
    # full cutout mask field, packed like images: [n, 128, 2*W]
    full = np.maximum(prm["my"][:, :, None], prm["mx"][:, None, :])  # [n,h,w]
    mask_bf = np.ascontiguousarray(
        full.reshape(-1, 2, 128, W).transpose(0, 2, 1, 3).reshape(-1, 128, 2 * W)
    ).astype(BF16)
    in_maps = []
    for i in range(NCORES):
        lo, hi = i * NLOC, (i + 1) * NLOC
        sm = pack_smalls(
            prm["M3"][lo:hi], prm["bias_c"][lo:hi], prm["sigma"][lo:hi],
            prm["my"][lo:hi],
        )
        in_maps.append({
            "imgs": np.ascontiguousarray(imgs_bf[lo:hi]),
            "noise": np.ascontiguousarray(noise_bf[lo:hi]),
            "wmat": np.ascontiguousarray(w_bf[lo:hi]),
            "maskf": np.ascontiguousarray(mask_bf[lo:hi]),
            "smalls": sm,
        })
    return in_maps, prm


_NC_CACHE = {}


def run_on_hw(images, gates, gauss, unif, noise_img, hz_fbank, trace=False):
    from concourse.bass_utils import run_bass_kernel_spmd

    if "nc" not in _NC_CACHE:
        _NC_CACHE["nc"] = build_bass()
    nc = _NC_CACHE["nc"]
    in_maps, _ = _prep_in_maps(images, gates, gauss, unif, noise_img, hz_fbank)
    res = run_bass_kernel_spmd(
        nc, in_maps, core_ids=list(range(NCORES)), trace=trace
    )
    out = np.concatenate(
        [unpack_images(np.asarray(r["out"]).astype(np.float32))
         for r in res.results], axis=0
    )
    return out.astype(np.float32), res


def kernel(images, gates, gauss, unif, noise_img, hz_fbank):
    images = np.asarray(images, dtype=np.float32)
    gates = np.asarray(gates, dtype=np.float32)
    gauss = np.asarray(gauss, dtype=np.float32)
    unif = np.asarray(unif, dtype=np.float32)
    noise_img = np.asarray(noise_img, dtype=np.float32)
    hz_fbank = np.asarray(hz_fbank, dtype=np.float32)
    out, _ = run_on_hw(images, gates, gauss, unif, noise_img, hz_fbank,
                       trace=os.environ.get("ADAAUG_TRACE", "0") == "1")
    return out


# revision 13
# speedup vs baseline: 1.3115x; 1.3115x over previous
"""AdaAug augmentation pipeline on 8 TRN2 NeuronCores (pure data parallel).

Pipeline per sample: color transform (3x3 + bias) -> 43-tap separable wavelet
filter with reflect padding -> additive RGB noise -> cutout mask.

Strategy:
  - Host derives per-sample small parameters exactly as the reference does:
    color matrix M3/b, filter taps hz', cutout mask vectors, sigma.
  - The separable conv with reflect padding is expressed as two chained PE
    matmuls per channel using a per-sample 256x256 reflect-Toeplitz matrix W:
        out1 = img.T @ W   ([w, h'] layout, contracts h)
        out2 = out1.T @ W  ([h', w'] layout, contracts w)
    No transposes needed: using the data as lhsT flips the layout each stage.
  - The 3x3 channel mix is folded into stage 2 by scaling W with M_cc' and
    accumulating over c' in PSUM (WhS built on the otherwise idle GPSIMD).
    The color bias reduces to a per-channel constant b_c * (sum k)^2, applied
    in the final fused DVE op. Noise is added via sigma*I identity matmuls.
  - Final op per channel: out = (psum + bias_c) * cutout_mask, one fused
    scalar_tensor_tensor DVE op, PSUM -> SBUF, then DMA out.
  - All bulk tensors are packed partition-major on the host so every DMA is
    128 contiguous per-partition chunks.
"""

import os
import sys

import numpy as np

if "/opt/trn_rl_repo" not in sys.path:
    sys.path.insert(0, "/opt/trn_rl_repo")

import ml_dtypes

N, C, H, W = 64, 3, 256, 256
NCORES = 8
NLOC = N // NCORES
TAP, PAD = 43, 21
PI = float(np.pi)
BRIGHT_STD, CONTRAST_STD, HUE_MAX, SAT_STD = 0.2, 0.5, 1.0, 1.0
IMGFILTER_STD, NOISE_STD, CUTOUT_SIZE = 1.0, 0.1, 0.5
P_GATE = 1.0
SM = 24  # per-sample slot count in the smalls tensor

BF16 = ml_dtypes.bfloat16


# --------------------------------------------------------------------------
# Host-side per-sample parameter derivation (mirrors the reference math)
# --------------------------------------------------------------------------

def color_matrices(gates, gauss, unif):
    """Returns M3 [n,3,3] and bvec [n,3] (float64)."""
    g = gates.astype(np.float64)
    ga = gauss.astype(np.float64)
    u = unif.astype(np.float64)
    n = g.shape[0]
    I4 = np.eye(4)
    inv_sqrt3 = 1.0 / np.sqrt(3.0)
    v3 = np.full(3, inv_sqrt3)
    v4 = np.array([inv_sqrt3, inv_sqrt3, inv_sqrt3, 0.0])
    vv = np.outer(v4, v4)

    b = np.where(g[:, 0] < P_GATE, ga[:, 0] * BRIGHT_STD, 0.0)
    T = np.broadcast_to(I4, (n, 4, 4)).copy()
    T[:, :3, 3] = b[:, None]

    c = np.where(g[:, 1] < P_GATE, 2.0 ** (ga[:, 1] * CONTRAST_STD), 1.0)
    S = I4[None] * np.stack([c, c, c, np.ones_like(c)], axis=1)[:, :, None]
    Cm = S @ T

    i_lf = np.floor(u[:, 0] * 2.0)
    i_lf = np.where(g[:, 2] < P_GATE, i_lf, 0.0)
    Cm = (I4[None] - 2.0 * vv[None] * i_lf[:, None, None]) @ Cm

    theta = (u[:, 1] * 2.0 - 1.0) * PI * HUE_MAX
    theta = np.where(g[:, 3] < P_GATE, theta, 0.0)
    I3 = np.eye(3)
    K = np.array([[0.0, -inv_sqrt3, inv_sqrt3],
                  [inv_sqrt3, 0.0, -inv_sqrt3],
                  [-inv_sqrt3, inv_sqrt3, 0.0]])
    co, si = np.cos(theta), np.sin(theta)
    R3 = ((1.0 - co)[:, None, None] * np.outer(v3, v3)[None]
          + co[:, None, None] * I3[None] + si[:, None, None] * K[None])
    R4 = np.broadcast_to(I4, (n, 4, 4)).copy()
    R4[:, :3, :3] = R3
    Cm = R4 @ Cm

    s = np.where(g[:, 4] < P_GATE, 2.0 ** (ga[:, 2] * SAT_STD), 1.0)
    Cm = (vv[None] + (I4 - vv)[None] * s[:, None, None]) @ Cm
    return Cm[:, :3, :3], Cm[:, :3, 3]


def band_taps(gates, gauss, hz_fbank):
    """Returns hz' [n, 43] (float64)."""
    g = gates.astype(np.float64)
    ga = gauss.astype(np.float64)
    fb = hz_fbank.astype(np.float64)
    n = g.shape[0]
    num_bands = fb.shape[0]
    ep = np.array([10.0, 1.0, 1.0, 1.0]) / 13.0
    gg = np.ones((n, num_bands))
    for i in range(num_bands):
        t_i = 2.0 ** (ga[:, 3 + i] * IMGFILTER_STD)
        t_i = np.where(g[:, 5 + i] < P_GATE, t_i, 1.0)
        t = np.ones((n, num_bands))
        t[:, i] = t_i
        t = t / np.sqrt(np.sum(ep * t * t, axis=-1, keepdims=True))
        gg = gg * t
    return gg @ fb


def toeplitz_reflect(k):
    """W [256,256] such that (reflect-pad-21 conv k) == W.T @ x.  k: [43]."""
    Wm = np.zeros((H, H))
    j = np.arange(H)
    for t in range(TAP):
        m = j + t - PAD
        m = np.abs(m)
        m = np.where(m > H - 1, 2 * (H - 1) - m, m)
        np.add.at(Wm, (m, j), k[t])
    return Wm


def mask_vectors(gates, unif):
    """Exact f32 cutout indicator vectors mx, my: [n, 256] each (1.0 outside)."""
    g32 = gates.astype(np.float32)
    u32 = unif.astype(np.float32)
    size = np.where(g32[:, 10] < np.float32(P_GATE),
                    np.float32(CUTOUT_SIZE), np.float32(0.0)).astype(np.float32)
    half = (size * np.float32(0.5)).astype(np.float32)
    coord = ((np.arange(W, dtype=np.float32) + np.float32(0.5))
             / np.float32(W)).astype(np.float32)
    cx, cy = u32[:, 2], u32[:, 3]
    mx = (np.abs(coord[None, :] - cx[:, None]) >= half[:, None]).astype(np.float32)
    my = (np.abs(coord[None, :] - cy[:, None]) >= half[:, None]).astype(np.float32)
    return mx, my


def derive_params(gates, gauss, unif, hz_fbank):
    """All per-sample derived parameters for the device kernel."""
    n = gates.shape[0]
    M3, bvec = color_matrices(gates, gauss, unif)
    hz = band_taps(gates, gauss, hz_fbank)
    sk = hz.sum(axis=1)
    bias_c = bvec * (sk ** 2)[:, None]  # [n, 3]
    g32 = gates.astype(np.float32)
    sigma = np.where(g32[:, 9] < np.float32(P_GATE),
                     np.abs(gauss[:, 7].astype(np.float32)) * np.float32(NOISE_STD),
                     np.float32(0.0))
    mx, my = mask_vectors(gates, unif)
    Wmats = np.stack([toeplitz_reflect(hz[s]) for s in range(n)])
    return dict(M3=M3, bias_c=bias_c, sigma=sigma, mx=mx, my=my, Wmats=Wmats)


def pack_smalls(M3, bias_c, sigma, my):
    """[128, n*SM] f32, partition-replicated scalars + partition-indexed my."""
    n = M3.shape[0]
    sm = np.zeros((128, n * SM), dtype=np.float32)
    for s in range(n):
        base = s * SM
        sm[:, base:base + 9] = M3[s].reshape(-1).astype(np.float32)[None, :]
        sm[:, base + 9] = np.float32(sigma[s])
        sm[:, base + 10:base + 13] = bias_c[s].astype(np.float32)[None, :]
        for t in range(2):
            sm[:, base + 13 + t] = my[s][t * 128:(t + 1) * 128]
    return sm


def pack_images(x):
    """[n, 3, 256, 256] -> partition-major [n, 128, 1536]:
    buf[s, p, c*512 + t*256 + w] = x[s, c, t*128 + p, w]."""
    n = x.shape[0]
    return np.ascontiguousarray(
        x.reshape(n, C, 2, 128, W).transpose(0, 3, 1, 2, 4).reshape(n, 128, C * 2 * W)
    )


def unpack_images(buf):
    """Inverse of pack_images (for the f32 output)."""
    n = buf.shape[0]
    return np.ascontiguousarray(
        buf.reshape(n, 128, C, 2, W).transpose(0, 2, 3, 1, 4).reshape(n, C, H, W)
    )


def pack_wmats(Wm):
    """[n, 256, 256] -> [n, 128, 512]: buf[s, p, t*256 + j] = W[s, t*128+p, j]."""
    n = Wm.shape[0]
    return np.ascontiguousarray(
        Wm.reshape(n, 2, 128, H).transpose(0, 2, 1, 3).reshape(n, 128, 2 * H)
    )


# --------------------------------------------------------------------------
# Pure-numpy emulation of the device pipeline (for host-side validation)
# --------------------------------------------------------------------------

def emulate(images, noise_img, params, f32_sim=True):
    """Emulates the device computation (in f64 or with bf16 rounding)."""
    M3, bias_c, sigma = params["M3"], params["bias_c"], params["sigma"]
    mx, my, Wmats = params["mx"], params["my"], params["Wmats"]
    n = images.shape[0]
    out = np.zeros((n, C, H, W), dtype=np.float64)
    for s in range(n):
        Wm = Wmats[s]
        img = images[s].astype(np.float64)
        st1 = np.tensordot(img, Wm, axes=([1], [0]))  # [c, w, h']
        conv = np.tensordot(st1, Wm, axes=([1], [0]))  # [c, h', w']
        mixed = np.tensordot(M3[s], conv, axes=([1], [0]))
        mixed += bias_c[s][:, None, None]
        mixed += sigma[s] * noise_img[s].astype(np.float64)
        mask = np.maximum(my[s][:, None], mx[s][None, :])
        out[s] = mixed * mask[None]
    return out


# --------------------------------------------------------------------------
# Bass kernel builder
# --------------------------------------------------------------------------

def _legalize_waits(nc, max_keep=1):
    """Split multi-semaphore waits into standalone EventSemaphore instructions.

    The deployed walrus accepts at most one sync-wait command per engine
    instruction; Tile emits several. Hoisting extras onto preceding
    EventSemaphore instructions on the same engine queue is semantically
    identical (engines execute their stream in order)."""
    from concourse import mybir
    n_split = 0
    for f in nc.m.functions:
        for blk in f.blocks:
            out = []
            changed = False
            for inst in blk.instructions:
                si = inst.sync_info
                w = list(si.on_wait) if si is not None else []
                if len(w) > max_keep:
                    for extra in w[:-max_keep]:
                        ev = mybir.InstEventSemaphore(
                            name=f"evw_{n_split}", ins=[], outs=[])
                        ev.engine = inst.engine
                        ev.sync_info = mybir.SyncInfo(
                            on_wait=[extra], on_update=[])
                        out.append(ev)
                        n_split += 1
                    inst.sync_info = mybir.SyncInfo(
                        on_wait=w[-max_keep:], on_update=list(si.on_update))
                    changed = True
                out.append(inst)
            if changed:
                blk.instructions = out
    return nc


def _ap_key(arg, extras=()):
    """Identity key for a lowered matmul weights AP."""
    try:
        return (arg.memref, arg.offset, str(arg.ap), str(arg.dtype)) + tuple(
            str(e) for e in extras)
    except AttributeError:
        return None


def _dedupe_ldweights(nc):
    """Drop InstLdweights whose weights AP is identical to the previous weight
    load in the final PE stream (only matmuls/waits in between). The PE array
    already holds those weights; sem waits/updates are preserved on a
    standalone EventSemaphore."""
    from concourse import mybir
    n_removed = 0
    for f in nc.m.functions:
        for blk in f.blocks:
            out = []
            changed = False
            last_key = None
            for inst in blk.instructions:
                if inst.engine == mybir.EngineType.PE:
                    if isinstance(inst, mybir.InstLdweights):
                        key = _ap_key(
                            inst.ins[0],
                            extras=(inst.perf_mode, inst.is_transpose,
                                    inst.tile_position),
                        )
                        if key is not None and key == last_key:
                            si = inst.sync_info
                            if si is not None and (list(si.on_wait)
                                                   or list(si.on_update)):
                                ev = mybir.InstEventSemaphore(
                                    name=f"ldw_ev_{n_removed}", ins=[], outs=[])
                                ev.engine = inst.engine
                                ev.sync_info = si
                                out.append(ev)
                            n_removed += 1
                            changed = True
                            continue
                        last_key = key
                    elif isinstance(inst, mybir.InstMatmult):
                        if inst.ldweights:
                            last_key = None  # self-loading matmul clobbers
                    elif isinstance(inst, mybir.InstEventSemaphore):
                        pass  # does not touch the PE array
                    else:
                        last_key = None  # unknown PE inst: invalidate
                out.append(inst)
            if changed:
                blk.instructions = out
    return n_removed


def build_bass(legalize=True, dedupe_ldw=True):
    import concourse.bass as bass
    import concourse.tile as tile
    from concourse import mybir

    f32 = mybir.dt.float32
    bf16 = mybir.dt.bfloat16
    Alu = mybir.AluOpType
    Act = mybir.ActivationFunctionType

    nc = bass.Bass()
    d_img = nc.declare_dram_parameter("imgs", [NLOC, 128, C * 2 * W], bf16,
                                      isOutput=False)
    d_noi = nc.declare_dram_parameter("noise", [NLOC, 128, C * 2 * W], bf16,
                                      isOutput=False)
    d_w = nc.declare_dram_parameter("wmat", [NLOC, 128, 2 * H], bf16,
                                    isOutput=False)
    d_mask = nc.declare_dram_parameter("maskf", [NLOC, 128, 2 * W], bf16,
                                       isOutput=False)
    d_sm = nc.declare_dram_parameter("smalls", [128, NLOC * SM], f32,
                                     isOutput=False)
    d_id = nc.declare_dram_parameter("consts", [128, 128], bf16, isOutput=False)
    d_out = nc.declare_dram_parameter("out", [NLOC, 128, C * 2 * W], bf16,
                                      isOutput=True)

    with tile.TileContext(nc) as tc:
        with (
            tc.tile_pool(name="singles", bufs=1) as singles,
            tc.tile_pool(name="io", bufs=4) as io,
            tc.tile_pool(name="work", bufs=3) as work,
            tc.tile_pool(name="ps1", bufs=4, space="PSUM") as ps1p,
            tc.tile_pool(name="ps2", bufs=4, space="PSUM") as ps2p,
        ):
            scal = singles.tile([128, NLOC * SM], f32)
            nc.sync.dma_start(out=scal, in_=d_sm[:])
            ident = singles.tile([128, 128], bf16)
            nc.sync.dma_start(out=ident, in_=d_id[:])

            def sc(s, k):
                return scal[:, s * SM + k: s * SM + k + 1]

            for s in range(NLOC):
                # ---- loads (all contiguous per partition) ----
                img_sb = io.tile([128, C * 2 * W], bf16, tag="img")
                nc.sync.dma_start(out=img_sb, in_=d_img[s])
                w_sb = io.tile([128, 2 * H], bf16, tag="wm")
                nc.sync.dma_start(out=w_sb, in_=d_w[s])

                # ---- stage 1: vertical conv, per input channel ----
                # psum1_cp[p, m*256 + h'] = sum_h img[cp, h, w=m*128+p] * W[h, h']
                ps1_t = [
                    ps1p.tile([128, 2 * H], mybir.dt.float32, tag="ps1",
                              name=f"ps1_{cp}")
                    for cp in range(C)
                ]
                for cp in range(C):
                    pt = ps1_t[cp]
                    for m in range(2):
                        for k in range(2):
                            nc.tensor.matmul(
                                pt[:, m * 256:(m + 1) * 256],
                                img_sb[:, cp * 512 + k * 256 + m * 128:
                                       cp * 512 + k * 256 + m * 128 + 128],
                                w_sb[:, k * 256:(k + 1) * 256],
                                start=(k == 0),
                                stop=(k == 1),
                            )
                out1 = work.tile([128, C * 2 * H], bf16, tag="out1")
                for cp in range(C):
                    nc.scalar.copy(out1[:, cp * 512:(cp + 1) * 512], ps1_t[cp])

                # ---- other loads + per-sample derived tiles ----
                noi_sb = io.tile([128, C * 2 * W], bf16, tag="noi")
                nc.sync.dma_start(out=noi_sb, in_=d_noi[s])
                maskS = io.tile([128, 2 * W], bf16, tag="mask")
                nc.sync.dma_start(out=maskS, in_=d_mask[s])
                whs = work.tile([128, 9 * 2 * H], bf16, tag="whs")
                for cc in range(9):
                    nc.vector.tensor_scalar_mul(
                        whs[:, cc * 512:(cc + 1) * 512], w_sb[:], sc(s, cc)
                    )
                sigI = work.tile([128, 128], bf16, tag="sigI")
                nc.scalar.activation(sigI[:], ident, Act.Copy, scale=sc(s, 9))

                # ---- stage 2: horizontal conv + channel mix + noise ----
                ps2_t = [
                    ps2p.tile([128, 2 * W], mybir.dt.float32, tag="ps2",
                              name=f"ps2_{c}")
                    for c in range(C)
                ]
                for m in range(2):
                    for cp in range(C):
                        for k in range(2):
                            lhsT = out1[:, cp * 512 + k * 256 + m * 128:
                                        cp * 512 + k * 256 + m * 128 + 128]
                            for c in range(C):
                                nc.tensor.matmul(
                                    ps2_t[c][:, m * 256:(m + 1) * 256],
                                    lhsT,
                                    whs[:, (c * 3 + cp) * 512 + k * 256:
                                        (c * 3 + cp) * 512 + k * 256 + 256],
                                    start=(cp == 0 and k == 0),
                                    stop=False,
                                )
                    for c in range(C):
                        nc.tensor.matmul(
                            ps2_t[c][:, m * 256:(m + 1) * 256],
                            sigI[:],
                            noi_sb[:, c * 512 + m * 256: c * 512 + m * 256 + 256],
                            start=False,
                            stop=True,
                        )

                # ---- final fused: (psum + bias_c) * mask -> SBUF bf16 ----
                outS = work.tile([128, C * 2 * W], bf16, tag="outS")
                for c in range(C):
                    nc.vector.scalar_tensor_tensor(
                        out=outS[:, c * 512:(c + 1) * 512],
                        in0=ps2_t[c][:],
                        scalar=sc(s, 10 + c),
                        in1=maskS[:],
                        op0=Alu.add,
                        op1=Alu.mult,
                    )
                nc.sync.dma_start(out=d_out[s], in_=outS)
    if dedupe_ldw:
        n = _dedupe_ldweights(nc)
        if os.environ.get("ADAAUG_DEBUG"):
            print(f"deduped {n} LDWEIGHTS")
    return _legalize_waits(nc) if legalize else nc


# --------------------------------------------------------------------------
# Entry point
# --------------------------------------------------------------------------

def _prep_in_maps(images, gates, gauss, unif, noise_img, hz_fbank):
    prm = derive_params(gates, gauss, unif, hz_fbank)
    imgs_bf = pack_images(images).astype(BF16)
    noise_bf = pack_images(noise_img).astype(BF16)
    w_bf = pack_wmats(prm["Wmats"].astype(np.float32)).astype(BF16)
    # full cutout mask field, packed like images: [n, 128, 2*W]
    full = np.maximum(prm["my"][:, :, None], prm["mx"][:, None, :])  # [n,h,w]
    mask_bf = np.ascontiguousarray(
        full.reshape(-1, 2, 128, W).transpose(0, 2, 1, 3).reshape(-1, 128, 2 * W)
    ).astype(BF16)
    in_maps = []
    for i in range(NCORES):
        lo, hi = i * NLOC, (i + 1) * NLOC
        sm = pack_smalls(
            prm["M3"][lo:hi], prm["bias_c"][lo:hi], prm["sigma"][lo:hi],
            prm["my"][lo:hi],
        )
        in_maps.append({
            "imgs": np.ascontiguousarray(imgs_bf[lo:hi]),
            "noise": np.ascontiguousarray(noise_bf[lo:hi]),
            "wmat": np.ascontiguousarray(w_bf[lo:hi]),
            "maskf": np.ascontiguousarray(mask_bf[lo:hi]),
            "smalls": sm,
            "consts": np.eye(128, dtype=BF16),
        })
    return in_maps, prm


_NC_CACHE = {}


def run_on_hw(images, gates, gauss, unif, noise_img, hz_fbank, trace=False):
    from concourse.bass_utils import run_bass_kernel_spmd

    if "nc" not in _NC_CACHE:
        _NC_CACHE["nc"] = build_bass()
    nc = _NC_CACHE["nc"]
    in_maps, _ = _prep_in_maps(images, gates, gauss, unif, noise_img, hz_fbank)
    res = run_bass_kernel_spmd(
        nc, in_maps, core_ids=list(range(NCORES)), trace=trace
    )
    out = np.concatenate(
        [unpack_images(np.asarray(r["out"]).astype(np.float32))
         for r in res.results], axis=0
    )
    return out.astype(np.float32), res


def kernel(images, gates, gauss, unif, noise_img, hz_fbank):
    images = np.asarray(images, dtype=np.float32)
    gates = np.asarray(gates, dtype=np.float32)
    gauss = np.asarray(gauss, dtype=np.float32)
    unif = np.asarray(unif, dtype=np.float32)
    noise_img = np.asarray(noise_img, dtype=np.float32)
    hz_fbank = np.asarray(hz_fbank, dtype=np.float32)
    out, _ = run_on_hw(images, gates, gauss, unif, noise_img, hz_fbank,
                       trace=os.environ.get("ADAAUG_TRACE", "0") == "1")
    return out
